# revision 7
# baseline (speedup 1.0000x reference)
"""Multi-head attention kernel for Trainium2, 8 NeuronCores — v2.

Problem: x [2, 2048, 1024], w_qkv [1024, 3072], w_proj [1024, 1024],
b_proj [1024] -> out [2, 2048, 1024]  (16 heads, head_dim 64, eval mode).

Sharding: core c -> batch b = c//4, head-group g = c%4 (4 heads).
Host sums the 4 partial proj outputs per batch and adds bias.

v2 changes vs baseline:
 - v computed directly in [keys, hd] layout (lhsT = xT token chunk,
   rhs = w_v) -> no PE transposes, fewer psum copies.
 - exp split across ScalarE (exact, table) and VectorE (Schraudolph
   int16 bit-trick: bits = trunc(184.665*s + 16250.5) viewed as bf16;
   constant factor cancels in softmax; ~50% of tiles approx).
 - normalize: DVE reciprocal (psum row) + GpSimd partition_broadcast +
   single DVE tensor_tensor mult into outT.
 - DMA d-chunk pipelining so qkv starts before loads finish.
"""

import sys
from contextlib import ExitStack

import numpy as np

if "/opt/trn_rl_repo" not in sys.path:
    sys.path.insert(0, "/opt/trn_rl_repo")

import ml_dtypes
import concourse.bacc as bacc
import concourse.mybir as mybir
import concourse.tile as tile
from concourse.bass_utils import run_bass_kernel_spmd

F32 = mybir.dt.float32
BF16 = mybir.dt.bfloat16
I16 = mybir.dt.int16
AF = mybir.ActivationFunctionType
MULT = mybir.AluOpType.mult

B, N, D = 2, 2048, 1024
H, HD = 16, 64
SCALE = HD ** -0.5
NCORES = 8
GROUP = 4          # cores per batch
HC = H // GROUP    # heads per core = 4
DC = HC * HD       # qkv out-dim slice per core = 256
QI_W = 1024        # attention qi tile width (half the sequence)
NK = N // 128      # 16 key chunks
VS_W = HC * 65     # v_store width per kj chunk
# Schraudolph bf16 exp constants (tuned on the exact inputs)
EXP_A = 184.6650
EXP_B = 16250.5


def _dve_approx(kj, hh):
    """Which exp tiles go to DVE with approximate exp (rest: ACT exact).
    Measured on HW: 48 DVE tiles -> 362us vs 296us all-ACT. DVE ops pay a
    pipeline DRAIN (~op length again) and sit in the AV matmuls' critical
    path, so offloading exp to DVE loses despite the sim liking it."""
    return False


def _dedupe_ldweights(nc):
    """Remove consecutive duplicate PE Ldweights (same physical AP),
    migrating their semaphore waits/updates to the following Matmult.
    Safe here: no SBUF weight region is rewritten between two
    consecutive same-AP loads (weights/x loaded once; v_store written
    before first use and only re-read afterwards within an iteration)."""
    PE = mybir.EngineType.PE
    removed = 0
    for f in nc.m.functions:
        for blk in f.blocks:
            out, cur = [], None
            pend_w, pend_u = [], []
            for inst in blk.instructions:
                if getattr(inst, "engine", None) == PE:
                    op = inst.opcode
                    if op == "Ldweights":
                        key = str(inst.ins[0])
                        si = inst.sync_info
                        if key == cur and not pend_w and not pend_u:
                            if si is not None:
                                pend_w += list(si.on_wait)
                                pend_u += list(si.on_update)
                            removed += 1
                            continue
                        cur = key
                    elif op == "Matmult":
                        if inst.is_transpose or inst.ldweights:
                            cur = None
                        if pend_w or pend_u:
                            si = inst.sync_info
                            if si is None:
                                inst.sync_info = mybir.SyncInfo(
                                    on_wait=pend_w, on_update=pend_u)
                            else:
                                si.on_wait = list(si.on_wait) + pend_w
                                si.on_update = list(si.on_update) + pend_u
                            pend_w, pend_u = [], []
                    else:
                        cur = None
                out.append(inst)
            assert not pend_w and not pend_u
            blk.instructions = out
    return removed


def _build_program(iters=1, num_devices=NCORES):
    nc = bacc.Bacc("TRN2", target_bir_lowering=False, debug=False,
                   num_devices=num_devices)
    xT = nc.dram_tensor("xT", [D, N], BF16, kind="ExternalInput").ap()
    wqkv = nc.dram_tensor("wqkv", [D, 3 * DC], BF16, kind="ExternalInput").ap()
    wproj = nc.dram_tensor("wproj", [DC, D], BF16, kind="ExternalInput").ap()
    y = nc.dram_tensor("y", [N, D], F32, kind="ExternalOutput").ap()

    with tile.TileContext(nc) as tc, ExitStack() as ctx:
        pools = _make_pools(tc, ctx)
        for _ in range(iters):
            _emit(nc, tc, pools, xT, wqkv, wproj, y)
    _dedupe_ldweights(nc)
    nc.compile()
    return nc


def _make_pools(tc, ctx):
    p = {}
    p["const"] = ctx.enter_context(tc.tile_pool(name="const", bufs=1))
    p["xt"] = ctx.enter_context(tc.tile_pool(name="xt", bufs=8))
    p["wq"] = ctx.enter_context(tc.tile_pool(name="wq", bufs=8))
    p["qk"] = ctx.enter_context(tc.tile_pool(name="qk", bufs=4))
    p["vs"] = ctx.enter_context(tc.tile_pool(name="vs", bufs=1))
    p["expp"] = ctx.enter_context(tc.tile_pool(name="expp", bufs=6))
    p["outp"] = ctx.enter_context(tc.tile_pool(name="outp", bufs=2))
    p["nrm"] = ctx.enter_context(tc.tile_pool(name="nrm", bufs=4))
    p["wpj"] = ctx.enter_context(tc.tile_pool(name="wpj", bufs=2))
    p["ysb"] = ctx.enter_context(tc.tile_pool(name="ysb", bufs=2))
    # PSUM: sc 2x[128,1024] = 4 banks; av 2x[65,1024] = 4 banks
    p["scps"] = ctx.enter_context(tc.tile_pool(name="scps", bufs=2, space="PSUM"))
    p["avps"] = ctx.enter_context(tc.tile_pool(name="avps", bufs=2, space="PSUM"))
    return p


def _emit(nc, tc, pools, xT, wqkv, wproj, y):
    const = pools["const"]
    qk_p = pools["qk"]
    exp_p = pools["expp"]
    nrm_p = pools["nrm"]
    sc_ps = pools["scps"]
    av_ps = pools["avps"]

    # ---------------- constants ----------------
    ones_b = const.tile([128, 64], BF16)
    nc.vector.memset(ones_b[:], 1.0)

    # ---------------- loads (d-chunk pipelined) ----------------
    wq_sb, xt_sb = [], []
    for d in range(8):
        tw = pools["wq"].tile([128, 3 * DC], BF16, tag="wq", name=f"wq{d}")
        nc.sync.dma_start(tw[:], wqkv[d * 128:(d + 1) * 128, :])
        wq_sb.append(tw)
        tx = pools["xt"].tile([128, N], BF16, tag="xt", name=f"xt{d}")
        nc.sync.dma_start(tx[:], xT[d * 128:(d + 1) * 128, :])
        xt_sb.append(tx)
    wpj_sb = []
    for kd in range(2):
        t = pools["wpj"].tile([128, D], BF16, tag="wpj", name=f"wpj{kd}")
        nc.sync.dma_start(t[:], wproj[kd * 128:(kd + 1) * 128, :])
        wpj_sb.append(t)

    # v_store: per kj chunk, per head: 64 v columns + a ones column
    v_store = pools["vs"].tile([128, NK * VS_W], BF16)
    vview = v_store[:].rearrange("p (c h x) -> p c h x", c=NK, h=HC)
    nc.vector.tensor_copy(
        vview[:, :, :, 64:65],
        ones_b[:, 0:NK * HC].rearrange("p (c h x) -> p c h x", c=NK, x=1),
    )

    # ---------------- v projection, direct [keys, hd] layout ----------
    # Emitted FIRST so v_store (+ p=0 q/k below) is ready as early as
    # possible and the ACT exp stream can start while p=1 projections
    # still run on the PE. All phase-1 psum->sbuf copies go to DVE so
    # ACT does nothing but exp.
    # 4 parallel token-chunk chains across 4 psum banks so same-bank
    # accumulating matmuls are >= 4 apart (RMW serialization otherwise)
    for tq in range(NK // 4):
        vps = []
        for ci in range(4):
            pool = sc_ps if ci < 2 else av_ps
            vp = pool.tile([128, DC], F32, tag="sc" if ci < 2 else "av",
                           name=f"vps{ci}")
            vps.append(vp)
        for d in range(8):
            for ci in range(4):
                t0 = (4 * tq + ci) * 128
                nc.tensor.matmul(
                    vps[ci][:],
                    xt_sb[d][:, t0:t0 + 128],
                    wq_sb[d][:, 2 * DC:3 * DC],
                    start=(d == 0), stop=(d == 7))
        for ci in range(4):
            kj = 4 * tq + ci
            dst = v_store[:, kj * VS_W:(kj + 1) * VS_W]
            nc.vector.tensor_copy(
                dst.rearrange("p (h x) -> p h x", x=65)[:, :, 0:64],
                vps[ci][:].rearrange("p (h x) -> p h x", x=64))

    # ---------------- q/k projections (both pairs) ----------------
    # qT_p/kT_p: [128, N] bf16; head 2p on partitions 0:64, 2p+1 on 64:128.
    qkT = {}
    for p in range(2):
        for kind in range(2):  # 0=q, 1=k
            t = qk_p.tile([128, N], BF16, tag="qk", name=f"qk{p}{kind}")
            qkT[(p, kind)] = t
            off = kind * DC + p * 128
            ps0 = av_ps.tile([128, 512], F32, tag="av", name="mmq0")
            ps1 = av_ps.tile([128, 512], F32, tag="av", name="mmq1")
            ps2 = sc_ps.tile([128, 512], F32, tag="sc", name="mmq2")
            ps3 = sc_ps.tile([128, 512], F32, tag="sc", name="mmq3")
            chains = (ps0, ps1, ps2, ps3)
            for d in range(8):
                for nq, ps in enumerate(chains):
                    nc.tensor.matmul(
                        ps[:],
                        wq_sb[d][:, off:off + 128],
                        xt_sb[d][:, nq * 512:(nq + 1) * 512],
                        start=(d == 0), stop=(d == 7))
            for nq, ps in enumerate(chains):
                nc.vector.tensor_copy(t[:, nq * 512:(nq + 1) * 512], ps[:])

    # ---------------- attention ----------------
    outT = []
    for i in range(2):
        t = pools["outp"].tile([128, N], BF16, tag="outT", name=f"outT{i}")
        outT.append(t)

    for p in range(2):
        qT, kT = qkT[(p, 0)], qkT[(p, 1)]
        for half in range(2):
            q0 = half * QI_W
            avA = av_ps.tile([65, QI_W], F32, tag="av", name="avA")
            avB = av_ps.tile([65, QI_W], F32, tag="av", name="avB")
            for kj in range(NK):
                scA = sc_ps.tile([128, QI_W], F32, tag="sc", name="scA")
                scB = sc_ps.tile([128, QI_W], F32, tag="sc", name="scB")
                # strict A,B,A,B alternation: every MM's row group differs
                # from its predecessor -> LDW pull-ahead + tile overlap
                for i in (0, 512):
                    nc.tensor.matmul(
                        scA[:, i:i + 512],
                        kT[0:64, kj * 128:(kj + 1) * 128],
                        qT[0:64, q0 + i:q0 + i + 512],
                        start=True, stop=True)
                    nc.tensor.matmul(
                        scB[:, i:i + 512],
                        kT[64:128, kj * 128:(kj + 1) * 128],
                        qT[64:128, q0 + i:q0 + i + 512],
                        start=True, stop=True)
                exs = []
                for hh, sc in ((0, scA), (1, scB)):
                    if _dve_approx(kj, hh):
                        ei = exp_p.tile([128, QI_W], I16, tag="exp",
                                        name=f"exi{hh}")
                        nc.vector.tensor_scalar(
                            ei[:], sc[:], EXP_A, EXP_B, MULT,
                            mybir.AluOpType.add)
                        exs.append((ei, True))
                    else:
                        eb = exp_p.tile([128, QI_W], BF16, tag="exp",
                                        name=f"exb{hh}")
                        nc.scalar.activation(eb[:], sc[:], AF.Exp)
                        exs.append((eb, False))
                for hh, av in ((0, avA), (1, avB)):
                    vc = kj * VS_W + (2 * p + hh) % HC * 65
                    ex_t, is_i16 = exs[hh]
                    for i in (0, 512):
                        rhs = ex_t[:, i:i + 512]
                        if is_i16:
                            rhs = rhs.bitcast(BF16)
                        nc.tensor.matmul(
                            av[:, i:i + 512],
                            v_store[:, vc:vc + 65],
                            rhs,
                            start=(kj == 0), stop=(kj == NK - 1))
            for hh, av in ((0, avA), (1, avB)):
                # one DVE copy frees the av PSUM bank pair for the next
                # segment; recip/broadcast/multiply run from SBUF, with the
                # multiply on the otherwise-idle Pool engine (outT is only
                # consumed by the proj phase at the end).
                avs = nrm_p.tile([65, QI_W], F32, tag="avs", name="avs")
                nc.vector.tensor_copy(avs[:], av[:])
                rc = nrm_p.tile([1, QI_W], F32, tag="rc", name="rc")
                nc.vector.reciprocal(rc[:], avs[64:65, :])
                bc = nrm_p.tile([64, QI_W], F32, tag="bc", name="bc")
                nc.gpsimd.partition_broadcast(bc[:], rc[:])
                nc.gpsimd.tensor_tensor(
                    outT[p][hh * 64:(hh + 1) * 64, q0:q0 + QI_W],
                    avs[0:64, :], bc[:], MULT)

    # ---------------- partial output projection ----------------
    for m2 in range(N // 256):
        ysbA = pools["ysb"].tile([128, D], F32, tag="ysb", name="ysbA")
        ysbB = pools["ysb"].tile([128, D], F32, tag="ysb", name="ysbB")
        ps0 = av_ps.tile([128, 512], F32, tag="av", name="mp0")
        ps1 = av_ps.tile([128, 512], F32, tag="av", name="mp1")
        ps2 = sc_ps.tile([128, 512], F32, tag="sc", name="mp2")
        ps3 = sc_ps.tile([128, 512], F32, tag="sc", name="mp3")
        mo = ((2 * m2, 0, ps0), (2 * m2, 1, ps1),
              (2 * m2 + 1, 0, ps2), (2 * m2 + 1, 1, ps3))
        for kd in range(2):
            for m, o, ps in mo:
                nc.tensor.matmul(
                    ps[:], outT[kd][:, m * 128:(m + 1) * 128],
                    wpj_sb[kd][:, o * 512:(o + 1) * 512],
                    start=(kd == 0), stop=(kd == 1))
        nc.vector.tensor_copy(ysbA[:, 0:512], ps0[:])
        nc.scalar.copy(ysbA[:, 512:1024], ps1[:])
        nc.vector.tensor_copy(ysbB[:, 0:512], ps2[:])
        nc.scalar.copy(ysbB[:, 512:1024], ps3[:])
        nc.sync.dma_start(y[2 * m2 * 128:(2 * m2 + 1) * 128, :], ysbA[:])
        nc.sync.dma_start(y[(2 * m2 + 1) * 128:(2 * m2 + 2) * 128, :], ysbB[:])


_NC_CACHE = None


def _get_program():
    global _NC_CACHE
    if _NC_CACHE is None:
        _NC_CACHE = _build_program()
    return _NC_CACHE


def shard_inputs(x, w_qkv, w_proj, b_proj):
    """Build the 8 per-core input maps."""
    x = np.asarray(x, dtype=np.float32)
    w_qkv = np.asarray(w_qkv, dtype=np.float32)
    w_proj = np.asarray(w_proj, dtype=np.float32)
    bf = ml_dtypes.bfloat16
    in_maps = []
    xTs = [np.ascontiguousarray(x[b].T).astype(bf) for b in range(B)]
    for c in range(NCORES):
        b, g = divmod(c, GROUP)
        wq = w_qkv[:, g * DC:(g + 1) * DC] * np.float32(SCALE)
        wk = w_qkv[:, D + g * DC: D + (g + 1) * DC]
        wv = w_qkv[:, 2 * D + g * DC: 2 * D + (g + 1) * DC]
        in_maps.append({
            "xT": xTs[b],
            "wqkv": np.ascontiguousarray(
                np.concatenate([wq, wk, wv], axis=1)).astype(bf),
            "wproj": np.ascontiguousarray(
                w_proj[g * DC:(g + 1) * DC, :]).astype(bf),
        })
    return in_maps


def kernel(x, w_qkv, w_proj, b_proj):
    nc = _get_program()
    in_maps = shard_inputs(x, w_qkv, w_proj, b_proj)
    br = run_bass_kernel_spmd(nc, in_maps, core_ids=list(range(NCORES)))
    b_proj = np.asarray(b_proj, dtype=np.float32)
    out = np.empty((B, N, D), dtype=np.float32)
    for b in range(B):
        acc = br.results[4 * b]["y"].copy()
        for g in range(1, GROUP):
            acc += br.results[4 * b + g]["y"]
        out[b] = acc + b_proj
    return out


if __name__ == "__main__":
    rng = np.random.default_rng(0)
    x = rng.standard_normal((B, N, D), dtype=np.float32)
    w_qkv = rng.standard_normal((D, 3 * D), dtype=np.float32) * D ** -0.5
    w_proj = rng.standard_normal((D, D), dtype=np.float32) * D ** -0.5
    b_proj = rng.standard_normal((D,), dtype=np.float32) * 0.01
    got = kernel(x=x, w_qkv=w_qkv, w_proj=w_proj, b_proj=b_proj)
    qkv = (x.reshape(B * N, D) @ w_qkv).reshape(B, N, 3, H, HD)
    qkv = np.transpose(qkv, (2, 0, 3, 1, 4))
    q, k, v = qkv[0], qkv[1], qkv[2]
    s = np.einsum("bhqd,bhkd->bhqk", q, k) * SCALE
    e = np.exp(s - s.max(-1, keepdims=True))
    a = e / e.sum(-1, keepdims=True)
    o = np.einsum("bhqk,bhkd->bhqd", a, v)
    o = np.transpose(o, (0, 2, 1, 3)).reshape(B, N, D)
    want = o @ w_proj + b_proj
    err = np.abs(got - want)
    rel = err.max() / np.abs(want).max()
    print(f"absmax {err.max():.4e} rel-vs-absmax {rel:.4e} "
          f"rms-rel {np.sqrt((err**2).mean() / (want**2).mean()):.4e}")



# revision 10
# speedup vs baseline: 1.1382x; 1.1382x over previous
"""Multi-head attention kernel for Trainium2, 8 NeuronCores — v2.

Problem: x [2, 2048, 1024], w_qkv [1024, 3072], w_proj [1024, 1024],
b_proj [1024] -> out [2, 2048, 1024]  (16 heads, head_dim 64, eval mode).

Sharding: core c -> batch b = c//4, head-group g = c%4 (4 heads).
Host sums the 4 partial proj outputs per batch and adds bias.

v2 changes vs baseline:
 - v computed directly in [keys, hd] layout (lhsT = xT token chunk,
   rhs = w_v) -> no PE transposes, fewer psum copies.
 - exp split across ScalarE (exact, table) and VectorE (Schraudolph
   int16 bit-trick: bits = trunc(184.665*s + 16250.5) viewed as bf16;
   constant factor cancels in softmax; ~50% of tiles approx).
 - normalize: DVE reciprocal (psum row) + GpSimd partition_broadcast +
   single DVE tensor_tensor mult into outT.
 - DMA d-chunk pipelining so qkv starts before loads finish.
"""

import sys
from contextlib import ExitStack

import numpy as np

if "/opt/trn_rl_repo" not in sys.path:
    sys.path.insert(0, "/opt/trn_rl_repo")

import ml_dtypes
import concourse.bacc as bacc
import concourse.mybir as mybir
import concourse.tile as tile
from concourse.bass_utils import run_bass_kernel_spmd

F32 = mybir.dt.float32
BF16 = mybir.dt.bfloat16
I16 = mybir.dt.int16
AF = mybir.ActivationFunctionType
MULT = mybir.AluOpType.mult

B, N, D = 2, 2048, 1024
H, HD = 16, 64
SCALE = HD ** -0.5
NCORES = 8
GROUP = 4          # cores per batch
HC = H // GROUP    # heads per core = 4
DC = HC * HD       # qkv out-dim slice per core = 256
QI_W = 1024        # attention qi tile width (half the sequence)
NK = N // 128      # 16 key chunks
VS_W = HC * 65     # v_store width per kj chunk
# Schraudolph bf16 exp constants (tuned on the exact inputs)
EXP_A = 184.6650
EXP_B = 16250.5


def _dve_approx(kj, hh):
    """Which exp tiles go to DVE with approximate exp (rest: ACT exact).
    Measured on HW: 48 DVE tiles -> 362us vs 296us all-ACT. DVE ops pay a
    pipeline DRAIN (~op length again) and sit in the AV matmuls' critical
    path, so offloading exp to DVE loses despite the sim liking it."""
    return False


def _dedupe_ldweights(nc):
    """Remove consecutive duplicate PE Ldweights (same physical AP),
    migrating their semaphore waits/updates to the following Matmult.
    Safe here: no SBUF weight region is rewritten between two
    consecutive same-AP loads (weights/x loaded once; v_store written
    before first use and only re-read afterwards within an iteration)."""
    PE = mybir.EngineType.PE
    removed = 0
    for f in nc.m.functions:
        for blk in f.blocks:
            out, cur = [], None
            pend_w, pend_u = [], []
            for inst in blk.instructions:
                if getattr(inst, "engine", None) == PE:
                    op = inst.opcode
                    if op == "Ldweights":
                        key = str(inst.ins[0])
                        si = inst.sync_info
                        if key == cur and not pend_w and not pend_u:
                            if si is not None:
                                pend_w += list(si.on_wait)
                                pend_u += list(si.on_update)
                            removed += 1
                            continue
                        cur = key
                    elif op == "Matmult":
                        if inst.is_transpose or inst.ldweights:
                            cur = None
                        if pend_w or pend_u:
                            si = inst.sync_info
                            if si is None:
                                inst.sync_info = mybir.SyncInfo(
                                    on_wait=pend_w, on_update=pend_u)
                            else:
                                si.on_wait = list(si.on_wait) + pend_w
                                si.on_update = list(si.on_update) + pend_u
                            pend_w, pend_u = [], []
                    else:
                        cur = None
                out.append(inst)
            assert not pend_w and not pend_u
            blk.instructions = out
    return removed


def _build_program(iters=1, num_devices=NCORES):
    nc = bacc.Bacc("TRN2", target_bir_lowering=False, debug=False,
                   num_devices=num_devices)
    xT = nc.dram_tensor("xT", [D, N], BF16, kind="ExternalInput").ap()
    wqkv = nc.dram_tensor("wqkv", [D, 3 * DC], BF16, kind="ExternalInput").ap()
    wproj = nc.dram_tensor("wproj", [DC, D], BF16, kind="ExternalInput").ap()
    y = nc.dram_tensor("y", [N, D], F32, kind="ExternalOutput").ap()

    with tile.TileContext(nc) as tc, ExitStack() as ctx:
        pools = _make_pools(tc, ctx)
        for _ in range(iters):
            _emit(nc, tc, pools, xT, wqkv, wproj, y)
    _dedupe_ldweights(nc)
    nc.compile()
    return nc


def _make_pools(tc, ctx):
    p = {}
    p["const"] = ctx.enter_context(tc.tile_pool(name="const", bufs=1))
    p["xt"] = ctx.enter_context(tc.tile_pool(name="xt", bufs=8))
    p["wq"] = ctx.enter_context(tc.tile_pool(name="wq", bufs=8))
    p["qk"] = ctx.enter_context(tc.tile_pool(name="qk", bufs=4))
    p["vs"] = ctx.enter_context(tc.tile_pool(name="vs", bufs=1))
    p["expp"] = ctx.enter_context(tc.tile_pool(name="expp", bufs=6))
    p["outp"] = ctx.enter_context(tc.tile_pool(name="outp", bufs=2))
    p["nrm"] = ctx.enter_context(tc.tile_pool(name="nrm", bufs=4))
    p["wpj"] = ctx.enter_context(tc.tile_pool(name="wpj", bufs=2))
    p["ysb"] = ctx.enter_context(tc.tile_pool(name="ysb", bufs=2))
    # PSUM (8 banks): sc 2x[128,1024] = 4; av 1x[65,1024] = 2 (heads are
    # processed sequentially, one accumulator); ilv 2x[128,512] = 2 for
    # projection chains interleaved into the attention loop.
    p["scps"] = ctx.enter_context(tc.tile_pool(name="scps", bufs=2, space="PSUM"))
    p["avps"] = ctx.enter_context(tc.tile_pool(name="avps", bufs=1, space="PSUM"))
    p["ilvps"] = ctx.enter_context(tc.tile_pool(name="ilvps", bufs=2, space="PSUM"))
    return p


def _emit(nc, tc, pools, xT, wqkv, wproj, y):
    const = pools["const"]
    qk_p = pools["qk"]
    exp_p = pools["expp"]
    nrm_p = pools["nrm"]
    sc_ps = pools["scps"]
    av_ps = pools["avps"]
    ilv_ps = pools["ilvps"]

    # ---------------- constants ----------------
    ones_b = const.tile([128, 64], BF16)
    nc.vector.memset(ones_b[:], 1.0)

    # ---------------- loads (d-chunk pipelined) ----------------
    wq_sb, xt_sb = [], []
    for d in range(8):
        tw = pools["wq"].tile([128, 3 * DC], BF16, tag="wq", name=f"wq{d}")
        nc.sync.dma_start(tw[:], wqkv[d * 128:(d + 1) * 128, :])
        wq_sb.append(tw)
        tx = pools["xt"].tile([128, N], BF16, tag="xt", name=f"xt{d}")
        nc.sync.dma_start(tx[:], xT[d * 128:(d + 1) * 128, :])
        xt_sb.append(tx)
    wpj_sb = []
    for kd in range(2):
        t = pools["wpj"].tile([128, D], BF16, tag="wpj", name=f"wpj{kd}")
        nc.sync.dma_start(t[:], wproj[kd * 128:(kd + 1) * 128, :])
        wpj_sb.append(t)

    # v_store: per kj chunk, per head: 64 v columns + a ones column
    v_store = pools["vs"].tile([128, NK * VS_W], BF16)
    vview = v_store[:].rearrange("p (c h x) -> p c h x", c=NK, h=HC)
    nc.vector.tensor_copy(
        vview[:, :, :, 64:65],
        ones_b[:, 0:NK * HC].rearrange("p (c h x) -> p c h x", c=NK, x=1),
    )

    # ---------------- v projection, direct [keys, hd] layout ----------
    # Emitted first so attention p=0 only waits on v_store + p=0 q/k.
    # 4 parallel token-chunk chains across 4 psum banks so same-bank
    # accumulating matmuls are >= 4 apart (RMW serialization otherwise)
    _vpools = ((sc_ps, "sc"), (sc_ps, "sc"), (av_ps, "av"), (ilv_ps, "ilv"))
    for tq in range(NK // 4):
        vps = []
        for ci in range(4):
            pool, tag = _vpools[ci]
            vps.append(pool.tile([128, DC], F32, tag=tag, name=f"vps{ci}"))
        for d in range(8):
            for ci in range(4):
                t0 = (4 * tq + ci) * 128
                nc.tensor.matmul(
                    vps[ci][:],
                    xt_sb[d][:, t0:t0 + 128],
                    wq_sb[d][:, 2 * DC:3 * DC],
                    start=(d == 0), stop=(d == 7))
        for ci in range(4):
            kj = 4 * tq + ci
            dst = v_store[:, kj * VS_W:(kj + 1) * VS_W]
            if ci % 2 == 0:
                nc.vector.tensor_copy(
                    dst.rearrange("p (h x) -> p h x", x=65)[:, :, 0:64],
                    vps[ci][:].rearrange("p (h x) -> p h x", x=64))
            else:
                nc.scalar.copy(
                    dst.rearrange("p (h x) -> p h x", x=65)[:, :, 0:64],
                    vps[ci][:].rearrange("p (h x) -> p h x", x=64))

    # ---------------- q/k projections for p=0 only ----------------
    # qT_p/kT_p: [128, N] bf16; head 2p on partitions 0:64, 2p+1 on 64:128.
    # p=1 projection chains are interleaved into the p=0 attention loop
    # below (PE has slack there; the loop is ACT-exp bound).
    qkT = {}
    _qchains = ((av_ps, "av"), (ilv_ps, "ilv"), (sc_ps, "sc"), (sc_ps, "sc"))
    for kind in range(2):  # 0=q, 1=k
        t = qk_p.tile([128, N], BF16, tag="qk", name=f"qk0{kind}")
        qkT[(0, kind)] = t
        off = kind * DC
        chains = []
        for nq in range(4):
            pool, tag = _qchains[nq]
            chains.append(pool.tile([128, 512], F32, tag=tag, name=f"mmq{nq}"))
        for d in range(8):
            for nq, ps in enumerate(chains):
                nc.tensor.matmul(
                    ps[:],
                    wq_sb[d][:, off:off + 128],
                    xt_sb[d][:, nq * 512:(nq + 1) * 512],
                    start=(d == 0), stop=(d == 7))
        for nq, ps in enumerate(chains):
            if nq % 2 == 0:
                nc.vector.tensor_copy(t[:, nq * 512:(nq + 1) * 512], ps[:])
            else:
                nc.scalar.copy(t[:, nq * 512:(nq + 1) * 512], ps[:])
    for kind in range(2):
        qkT[(1, kind)] = qk_p.tile([128, N], BF16, tag="qk", name=f"qk1{kind}")

    # ---------------- attention ----------------
    outT = []
    for i in range(2):
        t = pools["outp"].tile([128, N], BF16, tag="outT", name=f"outT{i}")
        outT.append(t)

    # interleave schedule for p=1 q/k projections: one matmul per p=0
    # attention iteration; chain (kind, nq) accumulates over d=0..7 in a
    # 1-bank ilv psum tile, then a DVE copy moves it into qkT[(1, kind)].
    ilv_jobs = [(kind, nq, d)
                for kind in range(2) for nq in range(4) for d in range(8)]
    ilv_state = {"t": 0, "cur": None}

    def _ilv_step():
        t_i = ilv_state["t"]
        if t_i >= len(ilv_jobs):
            return
        ilv_state["t"] = t_i + 1
        kind, nq, d = ilv_jobs[t_i]
        if d == 0:
            ilv_state["cur"] = ilv_ps.tile([128, 512], F32, tag="ilv",
                                           name=f"ilv{kind}{nq}")
        cur = ilv_state["cur"]
        off = kind * DC + 128
        nc.tensor.matmul(
            cur[:],
            wq_sb[d][:, off:off + 128],
            xt_sb[d][:, nq * 512:(nq + 1) * 512],
            start=(d == 0), stop=(d == 7))
        if d == 7:
            nc.vector.tensor_copy(
                qkT[(1, kind)][:, nq * 512:(nq + 1) * 512], cur[:])

    for p in range(2):
        qT, kT = qkT[(p, 0)], qkT[(p, 1)]
        for half in range(2):
            q0 = half * QI_W
            for hh in range(2):
                r0 = hh * 64
                av = av_ps.tile([65, QI_W], F32, tag="av", name="av")
                for kj in range(NK):
                    sc = sc_ps.tile([128, QI_W], F32, tag="sc", name="sc")
                    for i in (0, 512):
                        nc.tensor.matmul(
                            sc[:, i:i + 512],
                            kT[r0:r0 + 64, kj * 128:(kj + 1) * 128],
                            qT[r0:r0 + 64, q0 + i:q0 + i + 512],
                            start=True, stop=True)
                    eb = exp_p.tile([128, QI_W], BF16, tag="exp", name="exb")
                    nc.scalar.activation(eb[:], sc[:], AF.Exp)
                    vc = kj * VS_W + (2 * p + hh) % HC * 65
                    for i in (0, 512):
                        nc.tensor.matmul(
                            av[:, i:i + 512],
                            v_store[:, vc:vc + 65],
                            eb[:, i:i + 512],
                            start=(kj == 0), stop=(kj == NK - 1))
                    if p == 0:
                        _ilv_step()
                # one DVE copy frees the av PSUM bank pair for the next
                # sweep; recip/broadcast/multiply run from SBUF, with the
                # multiply on the otherwise-idle Pool engine (outT is only
                # consumed by the proj phase at the end).
                avs = nrm_p.tile([65, QI_W], F32, tag="avs", name="avs")
                nc.vector.tensor_copy(avs[:], av[:])
                rc = nrm_p.tile([1, QI_W], F32, tag="rc", name="rc")
                nc.vector.reciprocal(rc[:], avs[64:65, :])
                bc = nrm_p.tile([64, QI_W], F32, tag="bc", name="bc")
                nc.gpsimd.partition_broadcast(bc[:], rc[:])
                nc.gpsimd.tensor_tensor(
                    outT[p][r0:r0 + 64, q0:q0 + QI_W],
                    avs[0:64, :], bc[:], MULT)

    # ---------------- partial output projection ----------------
    for m2 in range(N // 256):
        ysbA = pools["ysb"].tile([128, D], F32, tag="ysb", name="ysbA")
        ysbB = pools["ysb"].tile([128, D], F32, tag="ysb", name="ysbB")
        ps0 = av_ps.tile([128, 512], F32, tag="av", name="mp0")
        ps1 = ilv_ps.tile([128, 512], F32, tag="ilv", name="mp1")
        ps2 = sc_ps.tile([128, 512], F32, tag="sc", name="mp2")
        ps3 = sc_ps.tile([128, 512], F32, tag="sc", name="mp3")
        mo = ((2 * m2, 0, ps0), (2 * m2, 1, ps1),
              (2 * m2 + 1, 0, ps2), (2 * m2 + 1, 1, ps3))
        for kd in range(2):
            for m, o, ps in mo:
                nc.tensor.matmul(
                    ps[:], outT[kd][:, m * 128:(m + 1) * 128],
                    wpj_sb[kd][:, o * 512:(o + 1) * 512],
                    start=(kd == 0), stop=(kd == 1))
        nc.vector.tensor_copy(ysbA[:, 0:512], ps0[:])
        nc.scalar.copy(ysbA[:, 512:1024], ps1[:])
        nc.vector.tensor_copy(ysbB[:, 0:512], ps2[:])
        nc.scalar.copy(ysbB[:, 512:1024], ps3[:])
        nc.sync.dma_start(y[2 * m2 * 128:(2 * m2 + 1) * 128, :], ysbA[:])
        nc.sync.dma_start(y[(2 * m2 + 1) * 128:(2 * m2 + 2) * 128, :], ysbB[:])


_NC_CACHE = None


def _get_program():
    global _NC_CACHE
    if _NC_CACHE is None:
        _NC_CACHE = _build_program()
    return _NC_CACHE


def shard_inputs(x, w_qkv, w_proj, b_proj):
    """Build the 8 per-core input maps."""
    x = np.asarray(x, dtype=np.float32)
    w_qkv = np.asarray(w_qkv, dtype=np.float32)
    w_proj = np.asarray(w_proj, dtype=np.float32)
    bf = ml_dtypes.bfloat16
    in_maps = []
    xTs = [np.ascontiguousarray(x[b].T).astype(bf) for b in range(B)]
    for c in range(NCORES):
        b, g = divmod(c, GROUP)
        wq = w_qkv[:, g * DC:(g + 1) * DC] * np.float32(SCALE)
        wk = w_qkv[:, D + g * DC: D + (g + 1) * DC]
        wv = w_qkv[:, 2 * D + g * DC: 2 * D + (g + 1) * DC]
        in_maps.append({
            "xT": xTs[b],
            "wqkv": np.ascontiguousarray(
                np.concatenate([wq, wk, wv], axis=1)).astype(bf),
            "wproj": np.ascontiguousarray(
                w_proj[g * DC:(g + 1) * DC, :]).astype(bf),
        })
    return in_maps


def kernel(x, w_qkv, w_proj, b_proj):
    nc = _get_program()
    in_maps = shard_inputs(x, w_qkv, w_proj, b_proj)
    br = run_bass_kernel_spmd(nc, in_maps, core_ids=list(range(NCORES)))
    b_proj = np.asarray(b_proj, dtype=np.float32)
    out = np.empty((B, N, D), dtype=np.float32)
    for b in range(B):
        acc = br.results[4 * b]["y"].copy()
        for g in range(1, GROUP):
            acc += br.results[4 * b + g]["y"]
        out[b] = acc + b_proj
    return out


if __name__ == "__main__":
    rng = np.random.default_rng(0)
    x = rng.standard_normal((B, N, D), dtype=np.float32)
    w_qkv = rng.standard_normal((D, 3 * D), dtype=np.float32) * D ** -0.5
    w_proj = rng.standard_normal((D, D), dtype=np.float32) * D ** -0.5
    b_proj = rng.standard_normal((D,), dtype=np.float32) * 0.01
    got = kernel(x=x, w_qkv=w_qkv, w_proj=w_proj, b_proj=b_proj)
    qkv = (x.reshape(B * N, D) @ w_qkv).reshape(B, N, 3, H, HD)
    qkv = np.transpose(qkv, (2, 0, 3, 1, 4))
    q, k, v = qkv[0], qkv[1], qkv[2]
    s = np.einsum("bhqd,bhkd->bhqk", q, k) * SCALE
    e = np.exp(s - s.max(-1, keepdims=True))
    a = e / e.sum(-1, keepdims=True)
    o = np.einsum("bhqk,bhkd->bhqd", a, v)
    o = np.transpose(o, (0, 2, 1, 3)).reshape(B, N, D)
    want = o @ w_proj + b_proj
    err = np.abs(got - want)
    rel = err.max() / np.abs(want).max()
    print(f"absmax {err.max():.4e} rel-vs-absmax {rel:.4e} "
          f"rms-rel {np.sqrt((err**2).mean() / (want**2).mean()):.4e}")



# revision 14
# speedup vs baseline: 1.1755x; 1.0327x over previous
"""Multi-head attention kernel for Trainium2, 8 NeuronCores — v2.

Problem: x [2, 2048, 1024], w_qkv [1024, 3072], w_proj [1024, 1024],
b_proj [1024] -> out [2, 2048, 1024]  (16 heads, head_dim 64, eval mode).

Sharding: core c -> batch b = c//4, head-group g = c%4 (4 heads).
Host sums the 4 partial proj outputs per batch and adds bias.

v2 changes vs baseline:
 - v computed directly in [keys, hd] layout (lhsT = xT token chunk,
   rhs = w_v) -> no PE transposes, fewer psum copies.
 - exp split across ScalarE (exact, table) and VectorE (Schraudolph
   int16 bit-trick: bits = trunc(184.665*s + 16250.5) viewed as bf16;
   constant factor cancels in softmax; ~50% of tiles approx).
 - normalize: DVE reciprocal (psum row) + GpSimd partition_broadcast +
   single DVE tensor_tensor mult into outT.
 - DMA d-chunk pipelining so qkv starts before loads finish.
"""

import sys
from contextlib import ExitStack

import numpy as np

if "/opt/trn_rl_repo" not in sys.path:
    sys.path.insert(0, "/opt/trn_rl_repo")

import ml_dtypes
import concourse.bacc as bacc
import concourse.mybir as mybir
import concourse.tile as tile
from concourse.bass_utils import run_bass_kernel_spmd

F32 = mybir.dt.float32
BF16 = mybir.dt.bfloat16
I16 = mybir.dt.int16
AF = mybir.ActivationFunctionType
MULT = mybir.AluOpType.mult

B, N, D = 2, 2048, 1024
H, HD = 16, 64
SCALE = HD ** -0.5
NCORES = 8
GROUP = 4          # cores per batch
HC = H // GROUP    # heads per core = 4
DC = HC * HD       # qkv out-dim slice per core = 256
QI_W = 1024        # attention qi tile width (half the sequence)
NK = N // 128      # 16 key chunks
VS_W = HC * 65     # v_store width per kj chunk
# Schraudolph bf16 exp constants (tuned on the exact inputs)
EXP_A = 184.6650
EXP_B = 16250.5


def _dve_approx(kj, hh):
    """Which exp tiles go to DVE with approximate exp (rest: ACT exact).
    Measured on HW: 48 DVE tiles -> 362us vs 296us all-ACT. DVE ops pay a
    pipeline DRAIN (~op length again) and sit in the AV matmuls' critical
    path, so offloading exp to DVE loses despite the sim liking it."""
    return False


def _dedupe_ldweights(nc):
    """Remove consecutive duplicate PE Ldweights (same physical AP),
    migrating their semaphore waits/updates to the following Matmult.
    Safe here: no SBUF weight region is rewritten between two
    consecutive same-AP loads (weights/x loaded once; v_store written
    before first use and only re-read afterwards within an iteration)."""
    PE = mybir.EngineType.PE
    removed = 0
    for f in nc.m.functions:
        for blk in f.blocks:
            out, cur = [], None
            pend_w, pend_u = [], []
            for inst in blk.instructions:
                if getattr(inst, "engine", None) == PE:
                    op = inst.opcode
                    if op == "Ldweights":
                        key = str(inst.ins[0])
                        si = inst.sync_info
                        if key == cur and not pend_w and not pend_u:
                            if si is not None:
                                pend_w += list(si.on_wait)
                                pend_u += list(si.on_update)
                            removed += 1
                            continue
                        cur = key
                    elif op == "Matmult":
                        if inst.is_transpose or inst.ldweights:
                            cur = None
                        if pend_w or pend_u:
                            si = inst.sync_info
                            if si is None:
                                inst.sync_info = mybir.SyncInfo(
                                    on_wait=pend_w, on_update=pend_u)
                            else:
                                si.on_wait = list(si.on_wait) + pend_w
                                si.on_update = list(si.on_update) + pend_u
                            pend_w, pend_u = [], []
                    else:
                        cur = None
                out.append(inst)
            assert not pend_w and not pend_u
            blk.instructions = out
    return removed


def _build_program(iters=1, num_devices=NCORES):
    nc = bacc.Bacc("TRN2", target_bir_lowering=False, debug=False,
                   num_devices=num_devices)
    xT = nc.dram_tensor("xT", [D, N], BF16, kind="ExternalInput").ap()
    wqkv = nc.dram_tensor("wqkv", [D, 3 * DC], BF16, kind="ExternalInput").ap()
    wproj = nc.dram_tensor("wproj", [DC, D], BF16, kind="ExternalInput").ap()
    # y in bf16: halves the 8MB/core output DMA (host upcasts + sums the
    # partials in fp32; adds ~0.3% partial rounding error, within budget)
    y = nc.dram_tensor("y", [N, D], BF16, kind="ExternalOutput").ap()

    with tile.TileContext(nc) as tc, ExitStack() as ctx:
        pools = _make_pools(tc, ctx)
        for _ in range(iters):
            _emit(nc, tc, pools, xT, wqkv, wproj, y)
    _dedupe_ldweights(nc)
    nc.compile()
    return nc


def _make_pools(tc, ctx):
    p = {}
    p["const"] = ctx.enter_context(tc.tile_pool(name="const", bufs=1))
    p["xt"] = ctx.enter_context(tc.tile_pool(name="xt", bufs=8))
    p["wq"] = ctx.enter_context(tc.tile_pool(name="wq", bufs=8))
    p["qk"] = ctx.enter_context(tc.tile_pool(name="qk", bufs=4))
    p["vs"] = ctx.enter_context(tc.tile_pool(name="vs", bufs=1))
    p["expp"] = ctx.enter_context(tc.tile_pool(name="expp", bufs=6))
    p["outp"] = ctx.enter_context(tc.tile_pool(name="outp", bufs=2))
    p["nrm"] = ctx.enter_context(tc.tile_pool(name="nrm", bufs=4))
    p["wpj"] = ctx.enter_context(tc.tile_pool(name="wpj", bufs=2))
    p["ysb"] = ctx.enter_context(tc.tile_pool(name="ysb", bufs=2))
    # PSUM: sc 2x[128,1024] = 4 banks; av 2x[65,1024] = 4 banks
    p["scps"] = ctx.enter_context(tc.tile_pool(name="scps", bufs=2, space="PSUM"))
    p["avps"] = ctx.enter_context(tc.tile_pool(name="avps", bufs=2, space="PSUM"))
    return p


def _emit(nc, tc, pools, xT, wqkv, wproj, y):
    const = pools["const"]
    qk_p = pools["qk"]
    exp_p = pools["expp"]
    nrm_p = pools["nrm"]
    sc_ps = pools["scps"]
    av_ps = pools["avps"]

    # ---------------- constants ----------------
    ones_b = const.tile([128, 64], BF16)
    nc.vector.memset(ones_b[:], 1.0)

    # ---------------- loads (d-chunk pipelined) ----------------
    wq_sb, xt_sb = [], []
    for d in range(8):
        tw = pools["wq"].tile([128, 3 * DC], BF16, tag="wq", name=f"wq{d}")
        nc.sync.dma_start(tw[:], wqkv[d * 128:(d + 1) * 128, :])
        wq_sb.append(tw)
        tx = pools["xt"].tile([128, N], BF16, tag="xt", name=f"xt{d}")
        nc.sync.dma_start(tx[:], xT[d * 128:(d + 1) * 128, :])
        xt_sb.append(tx)
    wpj_sb = []
    for kd in range(2):
        t = pools["wpj"].tile([128, D], BF16, tag="wpj", name=f"wpj{kd}")
        nc.sync.dma_start(t[:], wproj[kd * 128:(kd + 1) * 128, :])
        wpj_sb.append(t)

    # v_store: per kj chunk, per head: 64 v columns + a ones column
    v_store = pools["vs"].tile([128, NK * VS_W], BF16)
    vview = v_store[:].rearrange("p (c h x) -> p c h x", c=NK, h=HC)
    nc.vector.tensor_copy(
        vview[:, :, :, 64:65],
        ones_b[:, 0:NK * HC].rearrange("p (c h x) -> p c h x", c=NK, x=1),
    )

    # ---------------- q/k projections (both pairs) ----------------
    # qT_p/kT_p: [128, N] bf16; head 2p on partitions 0:64, 2p+1 on 64:128.
    qkT = {}
    for p in range(2):
        for kind in range(2):  # 0=q, 1=k
            t = qk_p.tile([128, N], BF16, tag="qk", name=f"qk{p}{kind}")
            qkT[(p, kind)] = t
            off = kind * DC + p * 128
            ps0 = av_ps.tile([128, 512], F32, tag="av", name="mmq0")
            ps1 = av_ps.tile([128, 512], F32, tag="av", name="mmq1")
            ps2 = sc_ps.tile([128, 512], F32, tag="sc", name="mmq2")
            ps3 = sc_ps.tile([128, 512], F32, tag="sc", name="mmq3")
            chains = (ps0, ps1, ps2, ps3)
            for d in range(8):
                for nq, ps in enumerate(chains):
                    nc.tensor.matmul(
                        ps[:],
                        wq_sb[d][:, off:off + 128],
                        xt_sb[d][:, nq * 512:(nq + 1) * 512],
                        start=(d == 0), stop=(d == 7))
            for nq, ps in enumerate(chains):
                if nq % 2 == 0:
                    nc.vector.tensor_copy(t[:, nq * 512:(nq + 1) * 512], ps[:])
                else:
                    nc.scalar.copy(t[:, nq * 512:(nq + 1) * 512], ps[:])

    # ---------------- v projection, direct [keys, hd] layout ----------
    # 4 parallel token-chunk chains across 4 psum banks so same-bank
    # accumulating matmuls are >= 4 apart (RMW serialization otherwise)
    for tq in range(NK // 4):
        vps = []
        for ci in range(4):
            pool = sc_ps if ci < 2 else av_ps
            vp = pool.tile([128, DC], F32, tag="sc" if ci < 2 else "av",
                           name=f"vps{ci}")
            vps.append(vp)
        for d in range(8):
            for ci in range(4):
                t0 = (4 * tq + ci) * 128
                nc.tensor.matmul(
                    vps[ci][:],
                    xt_sb[d][:, t0:t0 + 128],
                    wq_sb[d][:, 2 * DC:3 * DC],
                    start=(d == 0), stop=(d == 7))
        for ci in range(4):
            kj = 4 * tq + ci
            dst = v_store[:, kj * VS_W:(kj + 1) * VS_W]
            if ci % 2 == 0:
                nc.vector.tensor_copy(
                    dst.rearrange("p (h x) -> p h x", x=65)[:, :, 0:64],
                    vps[ci][:].rearrange("p (h x) -> p h x", x=64))
            else:
                nc.scalar.copy(
                    dst.rearrange("p (h x) -> p h x", x=65)[:, :, 0:64],
                    vps[ci][:].rearrange("p (h x) -> p h x", x=64))

    # ---------------- attention ----------------
    outT = []
    for i in range(2):
        t = pools["outp"].tile([128, N], BF16, tag="outT", name=f"outT{i}")
        outT.append(t)

    for p in range(2):
        qT, kT = qkT[(p, 0)], qkT[(p, 1)]
        for half in range(2):
            q0 = half * QI_W
            avA = av_ps.tile([65, QI_W], F32, tag="av", name="avA")
            avB = av_ps.tile([65, QI_W], F32, tag="av", name="avB")
            for kj in range(NK):
                scA = sc_ps.tile([128, QI_W], F32, tag="sc", name="scA")
                scB = sc_ps.tile([128, QI_W], F32, tag="sc", name="scB")
                # strict A,B,A,B alternation: every MM's row group differs
                # from its predecessor -> LDW pull-ahead + tile overlap
                for i in (0, 512):
                    nc.tensor.matmul(
                        scA[:, i:i + 512],
                        kT[0:64, kj * 128:(kj + 1) * 128],
                        qT[0:64, q0 + i:q0 + i + 512],
                        start=True, stop=True)
                    nc.tensor.matmul(
                        scB[:, i:i + 512],
                        kT[64:128, kj * 128:(kj + 1) * 128],
                        qT[64:128, q0 + i:q0 + i + 512],
                        start=True, stop=True)
                exs = []
                for hh, sc in ((0, scA), (1, scB)):
                    if _dve_approx(kj, hh):
                        ei = exp_p.tile([128, QI_W], I16, tag="exp",
                                        name=f"exi{hh}")
                        nc.vector.tensor_scalar(
                            ei[:], sc[:], EXP_A, EXP_B, MULT,
                            mybir.AluOpType.add)
                        exs.append((ei, True))
                    else:
                        eb = exp_p.tile([128, QI_W], BF16, tag="exp",
                                        name=f"exb{hh}")
                        nc.scalar.activation(eb[:], sc[:], AF.Exp)
                        exs.append((eb, False))
                for hh, av in ((0, avA), (1, avB)):
                    vc = kj * VS_W + (2 * p + hh) % HC * 65
                    ex_t, is_i16 = exs[hh]
                    for i in (0, 512):
                        rhs = ex_t[:, i:i + 512]
                        if is_i16:
                            rhs = rhs.bitcast(BF16)
                        nc.tensor.matmul(
                            av[:, i:i + 512],
                            v_store[:, vc:vc + 65],
                            rhs,
                            start=(kj == 0), stop=(kj == NK - 1))
            for hh, av in ((0, avA), (1, avB)):
                # one DVE copy frees the av PSUM bank pair for the next
                # segment; recip/broadcast/multiply run from SBUF, with the
                # multiply on the otherwise-idle Pool engine (outT is only
                # consumed by the proj phase at the end).
                avs = nrm_p.tile([65, QI_W], F32, tag="avs", name="avs")
                nc.vector.tensor_copy(avs[:], av[:])
                rc = nrm_p.tile([1, QI_W], F32, tag="rc", name="rc")
                nc.vector.reciprocal(rc[:], avs[64:65, :])
                bc = nrm_p.tile([64, QI_W], F32, tag="bc", name="bc")
                nc.gpsimd.partition_broadcast(bc[:], rc[:])
                nc.gpsimd.tensor_tensor(
                    outT[p][hh * 64:(hh + 1) * 64, q0:q0 + QI_W],
                    avs[0:64, :], bc[:], MULT)

    # ---------------- partial output projection ----------------
    for m2 in range(N // 256):
        ysbA = pools["ysb"].tile([128, D], BF16, tag="ysb", name="ysbA")
        ysbB = pools["ysb"].tile([128, D], BF16, tag="ysb", name="ysbB")
        ps0 = av_ps.tile([128, 512], F32, tag="av", name="mp0")
        ps1 = av_ps.tile([128, 512], F32, tag="av", name="mp1")
        ps2 = sc_ps.tile([128, 512], F32, tag="sc", name="mp2")
        ps3 = sc_ps.tile([128, 512], F32, tag="sc", name="mp3")
        mo = ((2 * m2, 0, ps0), (2 * m2, 1, ps1),
              (2 * m2 + 1, 0, ps2), (2 * m2 + 1, 1, ps3))
        for kd in range(2):
            for m, o, ps in mo:
                nc.tensor.matmul(
                    ps[:], outT[kd][:, m * 128:(m + 1) * 128],
                    wpj_sb[kd][:, o * 512:(o + 1) * 512],
                    start=(kd == 0), stop=(kd == 1))
        nc.vector.tensor_copy(ysbA[:, 0:512], ps0[:])
        nc.scalar.copy(ysbA[:, 512:1024], ps1[:])
        nc.vector.tensor_copy(ysbB[:, 0:512], ps2[:])
        nc.scalar.copy(ysbB[:, 512:1024], ps3[:])
        nc.sync.dma_start(y[2 * m2 * 128:(2 * m2 + 1) * 128, :], ysbA[:])
        nc.sync.dma_start(y[(2 * m2 + 1) * 128:(2 * m2 + 2) * 128, :], ysbB[:])


_NC_CACHE = None


def _get_program():
    global _NC_CACHE
    if _NC_CACHE is None:
        _NC_CACHE = _build_program()
    return _NC_CACHE


def shard_inputs(x, w_qkv, w_proj, b_proj):
    """Build the 8 per-core input maps."""
    x = np.asarray(x, dtype=np.float32)
    w_qkv = np.asarray(w_qkv, dtype=np.float32)
    w_proj = np.asarray(w_proj, dtype=np.float32)
    bf = ml_dtypes.bfloat16
    in_maps = []
    xTs = [np.ascontiguousarray(x[b].T).astype(bf) for b in range(B)]
    for c in range(NCORES):
        b, g = divmod(c, GROUP)
        wq = w_qkv[:, g * DC:(g + 1) * DC] * np.float32(SCALE)
        wk = w_qkv[:, D + g * DC: D + (g + 1) * DC]
        wv = w_qkv[:, 2 * D + g * DC: 2 * D + (g + 1) * DC]
        in_maps.append({
            "xT": xTs[b],
            "wqkv": np.ascontiguousarray(
                np.concatenate([wq, wk, wv], axis=1)).astype(bf),
            "wproj": np.ascontiguousarray(
                w_proj[g * DC:(g + 1) * DC, :]).astype(bf),
        })
    return in_maps


def kernel(x, w_qkv, w_proj, b_proj):
    nc = _get_program()
    in_maps = shard_inputs(x, w_qkv, w_proj, b_proj)
    br = run_bass_kernel_spmd(nc, in_maps, core_ids=list(range(NCORES)))
    b_proj = np.asarray(b_proj, dtype=np.float32)
    out = np.empty((B, N, D), dtype=np.float32)
    for b in range(B):
        acc = np.asarray(br.results[4 * b]["y"], dtype=np.float32)
        for g in range(1, GROUP):
            acc += np.asarray(br.results[4 * b + g]["y"], dtype=np.float32)
        out[b] = acc + b_proj
    return out


if __name__ == "__main__":
    rng = np.random.default_rng(0)
    x = rng.standard_normal((B, N, D), dtype=np.float32)
    w_qkv = rng.standard_normal((D, 3 * D), dtype=np.float32) * D ** -0.5
    w_proj = rng.standard_normal((D, D), dtype=np.float32) * D ** -0.5
    b_proj = rng.standard_normal((D,), dtype=np.float32) * 0.01
    got = kernel(x=x, w_qkv=w_qkv, w_proj=w_proj, b_proj=b_proj)
    qkv = (x.reshape(B * N, D) @ w_qkv).reshape(B, N, 3, H, HD)
    qkv = np.transpose(qkv, (2, 0, 3, 1, 4))
    q, k, v = qkv[0], qkv[1], qkv[2]
    s = np.einsum("bhqd,bhkd->bhqk", q, k) * SCALE
    e = np.exp(s - s.max(-1, keepdims=True))
    a = e / e.sum(-1, keepdims=True)
    o = np.einsum("bhqk,bhkd->bhqd", a, v)
    o = np.transpose(o, (0, 2, 1, 3)).reshape(B, N, D)
    want = o @ w_proj + b_proj
    err = np.abs(got - want)
    rel = err.max() / np.abs(want).max()
    print(f"absmax {err.max():.4e} rel-vs-absmax {rel:.4e} "
          f"rms-rel {np.sqrt((err**2).mean() / (want**2).mean()):.4e}")

